# revision 18
# baseline (speedup 1.0000x reference)
"""CAM (channel attention) module kernel for Trainium2 (Bass/Tile).

Reference computation (per batch b):
    energy  = x_b @ x_b.T                      # [C, C], contraction over N
    att     = softmax(rowmax(energy) - energy) # row-wise over last axis
    out     = att @ x_b                        # [C, N]
    y_b     = gamma * out + x_b

Sharding: data-parallel over B across 8 NeuronCores (B=32 -> 4 per core),
gamma replicated, full CxC attention per core.

Identity used: softmax(rowmax(E) - E)[i,j] = exp(mn[i] - E[i,j]) / Z[i]
with mn[i] = min_j E[i,j], Z[i] = sum_j exp(mn[i] - E[i,j])  (shift
invariance of softmax; exact).

Per-batch phases (P=128 partitions, CO=C/P=4, KC=N/P=32):
    load:  X [P, CO, N] f32 DMA in chunks; X16 bf16 cast on DVE per chunk
    A:     x-transposes (PE, from X16) -> ps_x PSUM -> xt bf16 (ScalarE
           evac); mm1 accumulates E = x x^T upper-tri blocks in PSUM
    mirror: E[jc,ic] = E[ic,jc].T for ic<jc (PE transpose via SBUF stage)
    softmax: mn row-min (DVE), tS = exp(mn-E) bf16 + Z fused (ScalarE),
           rg = gamma/Z (DVE)
    D:     tT[j,i] = tS[i,j] (PE transposes -> ScalarE evac, bf16)
    E:     mm2 out = tT.T @ X16 per 512-col block; evac split: ScalarE
           scales by rg (frees PSUM bank early), DVE adds f32 residual,
           DMA out

Cross-batch software pipeline (PE program order per steady-state batch):
    [trans(b) kc>=PRE interleaved with mm1(b)] -> mirror(b) ->
    [trans(b+1) kc<PRE : fills the softmax(b) latency] -> tT(b) -> mm2(b)
This keeps the PE busy through the softmax serial chain (no >3.4us idle,
HAM stays warm) and hides the X16 cast + DMA of b+1 under compute of b.
"""

import contextlib

import numpy as np

P = 128

_CACHE = {}


DEFAULT_OPTS = dict(
    pre=12,        # k-chunks of next batch's transposes emitted early
    xt_bufs=16,    # xT k-chunk SBUF tiles (>= pre + 2)
    o_bufs=6,      # output staging tiles
    cast_engine="vector",   # engine for f32->bf16 natural-layout cast
    evac_engine="scalar",   # engine for ps_x -> xt evacuation
    trans_src16=False,      # PE-transpose X16 (bf16) instead of X (f32)
    ts_bf16=True,           # tS (exp output) in bf16
    evac_split=True,        # ScalarE scales PSUM->SBUF, DVE adds residual
    timing_io=False,
)


def _build(Bs, C, N, use_f32r=False, reps=1, **opts):
    import concourse.bass as bass  # noqa: F401
    import concourse.tile as tile
    import concourse.mybir as mybir
    from concourse import bacc
    from concourse.masks import make_identity

    o = dict(DEFAULT_OPTS)
    o.update(opts)

    F32 = mybir.dt.float32
    BF16 = mybir.dt.bfloat16
    AF = mybir.ActivationFunctionType
    ALU = mybir.AluOpType
    AX = mybir.AxisListType

    assert C == 4 * P and N % 512 == 0
    CO = C // P          # i/j chunks of 128
    KC = N // P          # n chunks of 128 (contraction for energy)
    NF = N // 512        # n chunks of 512 (DMA / matmul-2 free dim)
    PRE = o["pre"]
    TSDT = BF16 if o["ts_bf16"] else F32

    nc = bacc.Bacc(None, target_bir_lowering=False, debug=False)
    x_in = nc.dram_tensor("x", [Bs, C, N], F32, kind="ExternalInput")
    g_in = nc.dram_tensor("gamma", [1], F32, kind="ExternalInput")
    y_out = nc.dram_tensor("y", [Bs, C, N], F32, kind="ExternalOutput")

    with tile.TileContext(nc) as tc:
        with (
            tc.tile_pool(name="consts", bufs=1) as consts,
            tc.tile_pool(name="xpool", bufs=2) as xpool,
            tc.tile_pool(name="x16pool", bufs=2 if o["trans_src16"] else 1)
            as x16pool,
            tc.tile_pool(name="xtp", bufs=o["xt_bufs"]) as xtp,
            tc.tile_pool(name="tpool", bufs=1) as tpool,
            tc.tile_pool(name="ttpool", bufs=1) as ttpool,
            tc.tile_pool(name="opool", bufs=o["o_bufs"]) as opool,
            tc.tile_pool(name="stats", bufs=2) as stats,
            tc.tile_pool(name="pe", bufs=1, space="PSUM") as psum_e,
            tc.tile_pool(name="pxt", bufs=2, space="PSUM") as psum_xt,
            tc.tile_pool(name="pacc", bufs=2, space="PSUM") as psum_acc,
        ):
            ident = consts.tile([P, P], F32)
            make_identity(nc, ident)
            ident16 = consts.tile([P, P], BF16)
            nc.vector.tensor_copy(ident16[:, :], ident[:, :])
            g_sb = consts.tile([1, 1], F32)
            nc.sync.dma_start(g_sb[:, :], g_in[:].rearrange("(a b) -> a b", a=1))
            g_col = consts.tile([P, 1], F32)
            nc.gpsimd.partition_broadcast(g_col[:, :], g_sb[:1, :1])

            def eng_copy(engine, out, in_):
                if engine == "vector":
                    nc.vector.tensor_copy(out, in_)
                elif engine == "scalar":
                    nc.scalar.copy(out, in_)
                else:
                    nc.gpsimd.tensor_copy(out, in_)

            # per-batch live tiles
            Xs, X16s, xts = {}, {}, {}

            def emit_dma(b):
                """DMA x_b in 512-col chunks."""
                x_b = x_in[b].rearrange("(co p) n -> p co n", p=P)
                X = xpool.tile([P, CO, N], F32, tag="X")
                Xs[b] = X
                # first 128 columns land alone so transposes start early
                nc.sync.dma_start(X[:, :, 0:P], x_b[:, :, 0:P])
                nc.sync.dma_start(X[:, :, P:512], x_b[:, :, P:512])
                for nf in range(1, NF):
                    s = slice(nf * 512, (nf + 1) * 512)
                    nc.sync.dma_start(X[:, :, s], x_b[:, :, s])

            def emit_cast(b):
                """f32->bf16 natural-layout copy for matmul-2's moving
                operand. Emitted at the start of batch b's own cycle: the
                single X16 buffer is free right then (mm2(b-1) just
                finished reading it), so these never head-of-line block
                the DVE queue."""
                X = Xs[b]
                X16 = x16pool.tile([P, CO, N], BF16, tag="X16")
                X16s[b] = X16
                # cast per (co, nf) piece: contiguous free dim -> DVE 2x mode
                for nf in range(NF):
                    s = slice(nf * 512, (nf + 1) * 512)
                    for co in range(CO):
                        eng_copy(o["cast_engine"], X16[:, co, s], X[:, co, s])

            def emit_trans(b, kc, evac=None):
                """Transpose the kc-th 128-col slab of x_b into xt bf16."""
                ks = slice(kc * P, (kc + 1) * P)
                src = X16s[b] if o["trans_src16"] else Xs[b]
                idt = ident16 if o["trans_src16"] else ident
                # pad bf16 tiles to a full 2KB PSUM bank so the two psx
                # ring buffers never share a bank (PE-W vs evac-R collide
                # at bank granularity)
                ps_x = psum_xt.tile(
                    [P, C], BF16 if o["trans_src16"] else F32,
                    tag="psx", name="ps_x",
                    padded_shape=[P, 1024] if o["trans_src16"] else None,
                )
                for co in range(CO):
                    nc.tensor.transpose(
                        ps_x[:, co * P:(co + 1) * P], src[:, co, ks], idt
                    )
                xt_k = xtp.tile([P, C], BF16, tag="xt", name="xt_k")
                eng_copy(evac or o["evac_engine"], xt_k[:, :], ps_x[:, :])
                xts[(b, kc)] = xt_k

            def emit_mm1(b, kc, E):
                xt_k = xts.pop((b, kc))
                for ic in range(CO):
                    nc.tensor.matmul(
                        E[:, ic, ic * P:],
                        xt_k[:, ic * P:(ic + 1) * P],
                        xt_k[:, ic * P:],
                        start=(kc == 0),
                        stop=(kc == KC - 1),
                    )

            def emit_mirror(b, E):
                # stage copies on DVE: ScalarE is busy with xt evacs at
                # the end of phase A, and the copies gate the mirror
                # matmuls which gate the row-min reads
                for jc in range(1, CO):
                    for ic in range(jc):
                        stg = xtp.tile([P, P], F32, tag="mirror_stage",
                                       bufs=2)
                        nc.vector.tensor_copy(
                            stg[:, :], E[:, ic, jc * P:(jc + 1) * P]
                        )
                        nc.tensor.matmul(
                            E[:, jc, ic * P:(ic + 1) * P],
                            stg[:, :],
                            ident,
                            is_transpose=True,
                            skip_group_check=True,
                        )

            def emit_softmax(b, E):
                mn = stats.tile([P, CO], F32, tag="mn")
                zs = stats.tile([P, CO], F32, tag="zs")
                rg = stats.tile([P, CO], F32, tag="rg")
                tS = tpool.tile([P, CO, C], TSDT, tag="t")
                for ic in range(CO):
                    nc.vector.tensor_reduce(
                        mn[:, ic:ic + 1], E[:, ic, :], AX.X, ALU.min
                    )
                for ic in range(CO):
                    nc.scalar.activation(
                        tS[:, ic, :], E[:, ic, :], AF.Exp,
                        bias=mn[:, ic:ic + 1], scale=-1.0,
                        accum_out=zs[:, ic:ic + 1],
                    )
                nc.vector.reciprocal(rg[:, :], zs[:, :])
                nc.vector.tensor_scalar_mul(rg[:, :], rg[:, :], g_col[:, :1])
                return tS, rg

            def emit_tT(b, tS):
                tT = ttpool.tile([P, CO, C], BF16, tag="tT")
                idt = ident16 if o["ts_bf16"] else ident
                for jc in range(CO):
                    ps_t = psum_acc.tile(
                        [P, C], TSDT, tag="acc", name="ps_t",
                        padded_shape=[P, 1024] if o["ts_bf16"] else None,
                    )
                    for ic in range(CO):
                        nc.tensor.transpose(
                            ps_t[:, ic * P:(ic + 1) * P],
                            tS[:, ic, jc * P:(jc + 1) * P],
                            idt,
                        )
                    nc.scalar.copy(tT[:, jc, :], ps_t[:, :])
                return tT

            def emit_mm2(b, tT, rg):
                X, X16 = Xs[b], X16s[b]
                y_b = y_out[b].rearrange("(co p) n -> p co n", p=P)
                for ic in range(CO):
                    for nf in range(NF):
                        ns = slice(nf * 512, (nf + 1) * 512)
                        ps2 = psum_acc.tile([P, C], F32, tag="acc")
                        for jc in range(CO):
                            nc.tensor.matmul(
                                ps2[:, :512],
                                tT[:, jc, ic * P:(ic + 1) * P],
                                X16[:, jc, ns],
                                start=(jc == 0),
                                stop=(jc == CO - 1),
                            )
                        ot = opool.tile([P, 512], F32, tag="o")
                        if o["evac_split"]:
                            nc.scalar.activation(
                                ot[:, :], ps2[:, :512], AF.Copy,
                                bias=0.0, scale=rg[:, ic:ic + 1],
                            )
                            nc.vector.tensor_add(
                                ot[:, :], ot[:, :], X[:, ic, ns]
                            )
                        else:
                            nc.vector.scalar_tensor_tensor(
                                ot[:, :], ps2[:, :512], rg[:, ic:ic + 1],
                                X[:, ic, ns],
                                op0=ALU.mult, op1=ALU.add,
                            )
                        nc.sync.dma_start(y_b[:, ic, ns], ot[:, :])
                del Xs[b], X16s[b]

            loop_ctx = (
                tc.For_i(0, reps, 1) if reps > 1 else contextlib.nullcontext()
            )
            with loop_ctx:
                emit_dma(0)
                for b in range(Bs):
                    first = (b == 0)
                    emit_cast(b)
                    E = psum_e.tile([P, CO, C], F32, tag="E")
                    if first:
                        # no prefetched transposes: run 2 ahead of mm1
                        emit_trans(b, 0)
                        emit_trans(b, 1)
                        for kc in range(KC):
                            if kc + 2 < KC:
                                emit_trans(b, kc + 2)
                            if kc == 16 and b + 1 < Bs:
                                emit_dma(b + 1)
                            emit_mm1(b, kc, E)
                    else:
                        # kc < PRE were transposed during softmax(b-1)
                        for kc in range(KC):
                            if kc == 0 and b + 1 < Bs:
                                emit_dma(b + 1)
                            if kc + PRE < KC:
                                emit_trans(b, kc + PRE)
                            emit_mm1(b, kc, E)
                    emit_mirror(b, E)
                    tS, rg = emit_softmax(b, E)
                    if b + 1 < Bs:
                        # alternate evac engine: ScalarE also runs the exp
                        # chain here, DVE runs the row-mins; splitting
                        # keeps the psx bank ping-pong fed from both
                        for kc in range(min(PRE, KC)):
                            emit_trans(
                                b + 1, kc,
                                evac="vector" if kc % 2 else "scalar",
                            )
                    tT = emit_tT(b, tS)
                    emit_mm2(b, tT, rg)

    nc.compile()
    return nc


def get_nc(Bs=4, C=512, N=4096, use_f32r=False, reps=1, **opts):
    key = (Bs, C, N, use_f32r, reps, tuple(sorted(opts.items())))
    if key not in _CACHE:
        _CACHE[key] = _build(Bs, C, N, use_f32r, reps, **opts)
    return _CACHE[key]


def kernel(x, gamma):
    """Full inputs in, full output out. x [32, 512, 4096] f32, gamma [1] f32."""
    from concourse.bass_utils import run_bass_kernel_spmd

    x = np.ascontiguousarray(np.asarray(x, dtype=np.float32))
    gamma = np.ascontiguousarray(np.asarray(gamma, dtype=np.float32))
    B, C, N = x.shape
    n_cores = 8
    assert B % n_cores == 0
    Bs = B // n_cores

    nc = get_nc(Bs, C, N)
    in_maps = [
        {"x": x[i * Bs:(i + 1) * Bs], "gamma": gamma} for i in range(n_cores)
    ]
    res = run_bass_kernel_spmd(nc, in_maps, core_ids=list(range(n_cores)))
    return np.concatenate([r["y"] for r in res.results], axis=0)


# revision 29
# speedup vs baseline: 1.2291x; 1.2291x over previous
"""CAM (channel attention) module kernel for Trainium2 (Bass/Tile).

Reference computation (per batch b):
    energy  = x_b @ x_b.T                      # [C, C], contraction over N
    att     = softmax(rowmax(energy) - energy) # row-wise over last axis
    out     = att @ x_b                        # [C, N]
    y_b     = gamma * out + x_b

Sharding: data-parallel over B across 8 NeuronCores (B=32 -> 4 per core),
gamma replicated, full CxC attention per core.

Identity used: softmax(rowmax(E) - E)[i,j] = exp(mn[i] - E[i,j]) / Z[i]
with mn[i] = min_j E[i,j], Z[i] = sum_j exp(mn[i] - E[i,j])  (shift
invariance of softmax; exact).

Per-batch phases (P=128 partitions, CO=C/P=4, KC=N/P=32):
    load:  X [P, CO, N] f32 DMA in chunks; X16 bf16 cast on DVE per chunk
    A:     x-transposes (PE, from X16) -> ps_x PSUM -> xt bf16 (ScalarE
           evac); mm1 accumulates E = x x^T upper-tri blocks in PSUM
    mirror: E[jc,ic] = E[ic,jc].T for ic<jc (PE transpose via SBUF stage)
    softmax: mn row-min (DVE), tS = exp(mn-E) bf16 + Z fused (ScalarE),
           rg = gamma/Z (DVE)
    D:     tT[j,i] = tS[i,j] (PE transposes -> ScalarE evac, bf16)
    E:     mm2 out = tT.T @ X16 per 512-col block; evac split: ScalarE
           scales by rg (frees PSUM bank early), DVE adds f32 residual,
           DMA out

Cross-batch software pipeline (PE program order per steady-state batch):
    [trans(b) kc>=PRE interleaved with mm1(b)] -> mirror(b) ->
    [trans(b+1) kc<PRE : fills the softmax(b) latency] -> tT(b) -> mm2(b)
This keeps the PE busy through the softmax serial chain (no >3.4us idle,
HAM stays warm) and hides the X16 cast + DMA of b+1 under compute of b.
"""

import contextlib

import numpy as np

P = 128

_CACHE = {}


DEFAULT_OPTS = dict(
    pre=8,         # k-chunks of next batch's transposes emitted early
    xt_bufs=9,     # xT k-chunk SBUF tiles (>= pre + 1)
    o_bufs=6,      # output staging tiles
    cast_engine="vector",   # engine for f32->bf16 natural-layout cast
    evac_engine="scalar",   # engine for ps_x -> xt evacuation
    bf_cut=10,     # kc >= bf_cut transpose X16 (bf16); below: X (f32)
    ts_bf16=True,           # tS (exp output) in bf16
    evac_split=True,        # ScalarE scales PSUM->SBUF, DVE adds residual
    timing_io=False,
)


def _build(Bs, C, N, use_f32r=False, reps=1, **opts):
    import concourse.bass as bass  # noqa: F401
    import concourse.tile as tile
    import concourse.mybir as mybir
    from concourse import bacc
    from concourse.masks import make_identity

    o = dict(DEFAULT_OPTS)
    o.update(opts)

    F32 = mybir.dt.float32
    BF16 = mybir.dt.bfloat16
    AF = mybir.ActivationFunctionType
    ALU = mybir.AluOpType
    AX = mybir.AxisListType

    assert C == 4 * P and N % 512 == 0
    CO = C // P          # i/j chunks of 128
    KC = N // P          # n chunks of 128 (contraction for energy)
    NF = N // 512        # n chunks of 512 (DMA / matmul-2 free dim)
    PRE = o["pre"]
    TSDT = BF16 if o["ts_bf16"] else F32

    nc = bacc.Bacc(None, target_bir_lowering=False, debug=False)
    x_in = nc.dram_tensor("x", [Bs, C, N], F32, kind="ExternalInput")
    g_in = nc.dram_tensor("gamma", [1], F32, kind="ExternalInput")
    y_out = nc.dram_tensor("y", [Bs, C, N], F32, kind="ExternalOutput")

    with tile.TileContext(nc) as tc:
        with (
            tc.tile_pool(name="consts", bufs=1) as consts,
            tc.tile_pool(name="xpool", bufs=2) as xpool,
            tc.tile_pool(name="x16pool", bufs=1) as x16pool,
            tc.tile_pool(name="xtp", bufs=o["xt_bufs"]) as xtp,
            tc.tile_pool(name="tpool", bufs=1) as tpool,
            tc.tile_pool(name="ttpool", bufs=1) as ttpool,
            tc.tile_pool(name="opool", bufs=o["o_bufs"]) as opool,
            tc.tile_pool(name="stats", bufs=1) as stats,
            tc.tile_pool(name="pe", bufs=1, space="PSUM") as psum_e,
            tc.tile_pool(name="pxt", bufs=2, space="PSUM") as psum_xt,
            tc.tile_pool(name="pacc", bufs=2, space="PSUM") as psum_acc,
        ):
            ident = consts.tile([P, P], F32)
            make_identity(nc, ident)
            ident16 = consts.tile([P, P], BF16)
            nc.vector.tensor_copy(ident16[:, :], ident[:, :])
            g_sb = consts.tile([1, 1], F32)
            nc.sync.dma_start(g_sb[:, :], g_in[:].rearrange("(a b) -> a b", a=1))
            g_col = consts.tile([P, 1], F32)
            nc.gpsimd.partition_broadcast(g_col[:, :], g_sb[:1, :1])

            def eng_copy(engine, out, in_):
                if engine == "vector":
                    nc.vector.tensor_copy(out, in_)
                elif engine == "scalar":
                    nc.scalar.copy(out, in_)
                else:
                    nc.gpsimd.tensor_copy(out, in_)

            # per-batch live tiles
            Xs, X16s, xts = {}, {}, {}

            def emit_dma(b):
                """DMA x_b in 512-col chunks."""
                x_b = x_in[b].rearrange("(co p) n -> p co n", p=P)
                X = xpool.tile([P, CO, N], F32, tag="X")
                Xs[b] = X
                # first 128 columns land alone so transposes start early
                nc.sync.dma_start(X[:, :, 0:P], x_b[:, :, 0:P])
                nc.sync.dma_start(X[:, :, P:512], x_b[:, :, P:512])
                for nf in range(1, NF):
                    s = slice(nf * 512, (nf + 1) * 512)
                    nc.sync.dma_start(X[:, :, s], x_b[:, :, s])

            def emit_cast(b):
                """f32->bf16 natural-layout copy: matmul-2's moving
                operand and the bf16-transpose source. Emitted at the
                start of batch b's own cycle: the single X16 buffer is
                free right then (mm2(b-1) just finished reading it), so
                these never head-of-line block the DVE queue. Chunk
                order starts at bf_cut's chunk so the first bf16
                transposes of phase A unblock earliest."""
                X = Xs[b]
                X16 = x16pool.tile([P, CO, N], BF16, tag="X16")
                X16s[b] = X16
                first_chunk = o["bf_cut"] // 4
                order = [(first_chunk + i) % NF for i in range(NF)]
                # cast per (co, nf) piece: contiguous free dim -> DVE 2x mode
                for nf in order:
                    s = slice(nf * 512, (nf + 1) * 512)
                    for co in range(CO):
                        eng_copy(o["cast_engine"], X16[:, co, s], X[:, co, s])

            def emit_trans(b, kc, evac=None):
                """Transpose the kc-th 128-col slab of x_b into xt bf16.
                kc >= bf_cut reads the bf16 X16 (faster PE transpose);
                below that reads X f32 (no cast dependency -- used by the
                cross-batch filler and the first A-phase groups)."""
                src16 = kc >= o["bf_cut"]
                ks = slice(kc * P, (kc + 1) * P)
                src = X16s[b] if src16 else Xs[b]
                idt = ident16 if src16 else ident
                # bf16 tiles padded to a full 2KB PSUM bank (same slot
                # size as the f32 ones) so the two psx ring buffers never
                # share a bank (PE-W vs evac-R collide at bank granularity)
                ps_x = psum_xt.tile(
                    [P, C], BF16 if src16 else F32,
                    tag="psx", name="ps_x",
                    padded_shape=[P, 1024] if src16 else None,
                )
                for co in range(CO):
                    nc.tensor.transpose(
                        ps_x[:, co * P:(co + 1) * P], src[:, co, ks], idt
                    )
                xt_k = xtp.tile([P, C], BF16, tag="xt", name="xt_k")
                eng_copy(evac or o["evac_engine"], xt_k[:, :], ps_x[:, :])
                xts[(b, kc)] = xt_k

            def emit_mm1(b, kc, E):
                xt_k = xts.pop((b, kc))
                for ic in range(CO):
                    nc.tensor.matmul(
                        E[:, ic, ic * P:],
                        xt_k[:, ic * P:(ic + 1) * P],
                        xt_k[:, ic * P:],
                        start=(kc == 0),
                        stop=(kc == KC - 1),
                    )

            def emit_mirror(b, E):
                for jc in range(1, CO):
                    for ic in range(jc):
                        stg = xtp.tile([P, P], F32, tag="mirror_stage",
                                       bufs=1)
                        nc.scalar.copy(
                            stg[:, :], E[:, ic, jc * P:(jc + 1) * P]
                        )
                        nc.tensor.matmul(
                            E[:, jc, ic * P:(ic + 1) * P],
                            stg[:, :],
                            ident,
                            is_transpose=True,
                            skip_group_check=True,
                        )

            def emit_softmax(b, E):
                mn = stats.tile([P, CO], F32, tag="mn")
                zs = stats.tile([P, CO], F32, tag="zs")
                rg = stats.tile([P, CO], F32, tag="rg")
                tS = tpool.tile([P, CO, C], TSDT, tag="t")
                for ic in range(CO):
                    nc.vector.tensor_reduce(
                        mn[:, ic:ic + 1], E[:, ic, :], AX.X, ALU.min
                    )
                for ic in range(CO):
                    nc.scalar.activation(
                        tS[:, ic, :], E[:, ic, :], AF.Exp,
                        bias=mn[:, ic:ic + 1], scale=-1.0,
                        accum_out=zs[:, ic:ic + 1],
                    )
                nc.vector.reciprocal(rg[:, :], zs[:, :])
                nc.vector.tensor_scalar_mul(rg[:, :], rg[:, :], g_col[:, :1])
                return tS, rg

            def emit_tT(b, tS):
                tT = ttpool.tile([P, CO, C], BF16, tag="tT")
                idt = ident16 if o["ts_bf16"] else ident
                for jc in range(CO):
                    ps_t = psum_acc.tile(
                        [P, C], TSDT, tag="acc", name="ps_t",
                        padded_shape=[P, 1024] if o["ts_bf16"] else None,
                    )
                    for ic in range(CO):
                        nc.tensor.transpose(
                            ps_t[:, ic * P:(ic + 1) * P],
                            tS[:, ic, jc * P:(jc + 1) * P],
                            idt,
                        )
                    nc.scalar.copy(tT[:, jc, :], ps_t[:, :])
                return tT

            def emit_mm2(b, tT, rg):
                X, X16 = Xs[b], X16s[b]
                y_b = y_out[b].rearrange("(co p) n -> p co n", p=P)
                for ic in range(CO):
                    for nf in range(NF):
                        ns = slice(nf * 512, (nf + 1) * 512)
                        ps2 = psum_acc.tile([P, C], F32, tag="acc")
                        for jc in range(CO):
                            nc.tensor.matmul(
                                ps2[:, :512],
                                tT[:, jc, ic * P:(ic + 1) * P],
                                X16[:, jc, ns],
                                start=(jc == 0),
                                stop=(jc == CO - 1),
                            )
                        ot = opool.tile([P, 512], F32, tag="o")
                        if o["evac_split"]:
                            nc.scalar.activation(
                                ot[:, :], ps2[:, :512], AF.Copy,
                                bias=0.0, scale=rg[:, ic:ic + 1],
                            )
                            nc.vector.tensor_add(
                                ot[:, :], ot[:, :], X[:, ic, ns]
                            )
                        else:
                            nc.vector.scalar_tensor_tensor(
                                ot[:, :], ps2[:, :512], rg[:, ic:ic + 1],
                                X[:, ic, ns],
                                op0=ALU.mult, op1=ALU.add,
                            )
                        nc.sync.dma_start(y_b[:, ic, ns], ot[:, :])
                del Xs[b], X16s[b]

            loop_ctx = (
                tc.For_i(0, reps, 1) if reps > 1 else contextlib.nullcontext()
            )
            with loop_ctx:
                emit_dma(0)
                for b in range(Bs):
                    first = (b == 0)
                    emit_cast(b)
                    E = psum_e.tile([P, CO, C], F32, tag="E")
                    if first:
                        # no prefetched transposes: run 2 ahead of mm1
                        emit_trans(b, 0)
                        emit_trans(b, 1)
                        for kc in range(KC):
                            if kc + 2 < KC:
                                emit_trans(b, kc + 2)
                            if kc == 16 and b + 1 < Bs:
                                emit_dma(b + 1)
                            emit_mm1(b, kc, E)
                    else:
                        # kc < PRE were transposed during softmax(b-1)
                        for kc in range(KC):
                            if kc == 0 and b + 1 < Bs:
                                emit_dma(b + 1)
                            if kc + PRE < KC:
                                emit_trans(b, kc + PRE)
                            emit_mm1(b, kc, E)
                    emit_mirror(b, E)
                    tS, rg = emit_softmax(b, E)
                    if b + 1 < Bs:
                        for kc in range(min(PRE, KC)):
                            emit_trans(b + 1, kc)
                    tT = emit_tT(b, tS)
                    emit_mm2(b, tT, rg)

    nc.compile()
    return nc


def get_nc(Bs=4, C=512, N=4096, use_f32r=False, reps=1, **opts):
    key = (Bs, C, N, use_f32r, reps, tuple(sorted(opts.items())))
    if key not in _CACHE:
        _CACHE[key] = _build(Bs, C, N, use_f32r, reps, **opts)
    return _CACHE[key]


def kernel(x, gamma):
    """Full inputs in, full output out. x [32, 512, 4096] f32, gamma [1] f32."""
    from concourse.bass_utils import run_bass_kernel_spmd

    x = np.ascontiguousarray(np.asarray(x, dtype=np.float32))
    gamma = np.ascontiguousarray(np.asarray(gamma, dtype=np.float32))
    B, C, N = x.shape
    n_cores = 8
    assert B % n_cores == 0
    Bs = B // n_cores

    nc = get_nc(Bs, C, N)
    in_maps = [
        {"x": x[i * Bs:(i + 1) * Bs], "gamma": gamma} for i in range(n_cores)
    ]
    res = run_bass_kernel_spmd(nc, in_maps, core_ids=list(range(n_cores)))
    return np.concatenate([r["y"] for r in res.results], axis=0)


# revision 30
# speedup vs baseline: 1.2712x; 1.0343x over previous
"""CAM (channel attention) module kernel for Trainium2 (Bass/Tile).

Reference computation (per batch b):
    energy  = x_b @ x_b.T                      # [C, C], contraction over N
    att     = softmax(rowmax(energy) - energy) # row-wise over last axis
    out     = att @ x_b                        # [C, N]
    y_b     = gamma * out + x_b

Sharding: data-parallel over B across 8 NeuronCores (B=32 -> 4 per core),
gamma replicated, full CxC attention per core.

Identity used: softmax(rowmax(E) - E)[i,j] = exp(mn[i] - E[i,j]) / Z[i]
with mn[i] = min_j E[i,j], Z[i] = sum_j exp(mn[i] - E[i,j])  (shift
invariance of softmax; exact).

Per-batch phases (P=128 partitions, CO=C/P=4, KC=N/P=32):
    load:  X [P, CO, N] f32 DMA in chunks; X16 bf16 cast on DVE per chunk
    A:     x-transposes (PE, from X16) -> ps_x PSUM -> xt bf16 (ScalarE
           evac); mm1 accumulates E = x x^T upper-tri blocks in PSUM
    mirror: E[jc,ic] = E[ic,jc].T for ic<jc (PE transpose via SBUF stage)
    softmax: mn row-min (DVE), tS = exp(mn-E) bf16 + Z fused (ScalarE),
           rg = gamma/Z (DVE)
    D:     tT[j,i] = tS[i,j] (PE transposes -> ScalarE evac, bf16)
    E:     mm2 out = tT.T @ X16 per 512-col block; evac split: ScalarE
           scales by rg (frees PSUM bank early), DVE adds f32 residual,
           DMA out

Cross-batch software pipeline (PE program order per steady-state batch):
    [trans(b) kc>=PRE interleaved with mm1(b)] -> mirror(b) ->
    [trans(b+1) kc<PRE : fills the softmax(b) latency] -> tT(b) -> mm2(b)
This keeps the PE busy through the softmax serial chain (no >3.4us idle,
HAM stays warm) and hides the X16 cast + DMA of b+1 under compute of b.
"""

import contextlib

import numpy as np

P = 128

_CACHE = {}


DEFAULT_OPTS = dict(
    pre=8,         # k-chunks of next batch's transposes emitted early
    xt_bufs=9,     # xT k-chunk SBUF tiles (>= pre + 1)
    o_bufs=6,      # output staging tiles
    cast_engine="vector",   # engine for f32->bf16 natural-layout cast
    evac_engine="scalar",   # engine for ps_x -> xt evacuation
    bf_cut=10,     # kc >= bf_cut transpose X16 (bf16); below: X (f32)
    ts_bf16=True,           # tS (exp output) in bf16
    evac_split=True,        # ScalarE scales PSUM->SBUF, DVE adds residual
    timing_io=False,
)


def _build(Bs, C, N, use_f32r=False, reps=1, **opts):
    import concourse.bass as bass  # noqa: F401
    import concourse.tile as tile
    import concourse.mybir as mybir
    from concourse import bacc
    from concourse.masks import make_identity

    o = dict(DEFAULT_OPTS)
    o.update(opts)

    F32 = mybir.dt.float32
    BF16 = mybir.dt.bfloat16
    AF = mybir.ActivationFunctionType
    ALU = mybir.AluOpType
    AX = mybir.AxisListType

    assert C == 4 * P and N % 512 == 0
    CO = C // P          # i/j chunks of 128
    KC = N // P          # n chunks of 128 (contraction for energy)
    NF = N // 512        # n chunks of 512 (DMA / matmul-2 free dim)
    PRE = o["pre"]
    TSDT = BF16 if o["ts_bf16"] else F32

    nc = bacc.Bacc(None, target_bir_lowering=False, debug=False)
    x_in = nc.dram_tensor("x", [Bs, C, N], F32, kind="ExternalInput")
    g_in = nc.dram_tensor("gamma", [1], F32, kind="ExternalInput")
    y_out = nc.dram_tensor("y", [Bs, C, N], F32, kind="ExternalOutput")

    with tile.TileContext(nc) as tc:
        with (
            tc.tile_pool(name="consts", bufs=1) as consts,
            tc.tile_pool(name="xpool", bufs=2) as xpool,
            tc.tile_pool(name="x16pool", bufs=1) as x16pool,
            tc.tile_pool(name="xtp", bufs=o["xt_bufs"]) as xtp,
            tc.tile_pool(name="tpool", bufs=1) as tpool,
            tc.tile_pool(name="ttpool", bufs=1) as ttpool,
            tc.tile_pool(name="opool", bufs=o["o_bufs"]) as opool,
            tc.tile_pool(name="stats", bufs=1) as stats,
            tc.tile_pool(name="pe", bufs=1, space="PSUM") as psum_e,
            tc.tile_pool(name="pxt", bufs=2, space="PSUM") as psum_xt,
            tc.tile_pool(name="pacc", bufs=2, space="PSUM") as psum_acc,
        ):
            ident = consts.tile([P, P], F32)
            make_identity(nc, ident)
            ident16 = consts.tile([P, P], BF16)
            nc.vector.tensor_copy(ident16[:, :], ident[:, :])
            g_sb = consts.tile([1, 1], F32)
            nc.sync.dma_start(g_sb[:, :], g_in[:].rearrange("(a b) -> a b", a=1))
            g_col = consts.tile([P, 1], F32)
            nc.gpsimd.partition_broadcast(g_col[:, :], g_sb[:1, :1])

            def eng_copy(engine, out, in_):
                if engine == "vector":
                    nc.vector.tensor_copy(out, in_)
                elif engine == "scalar":
                    nc.scalar.copy(out, in_)
                else:
                    nc.gpsimd.tensor_copy(out, in_)

            # per-batch live tiles
            Xs, X16s, xts = {}, {}, {}

            def emit_dma(b):
                """DMA x_b in 512-col chunks."""
                x_b = x_in[b].rearrange("(co p) n -> p co n", p=P)
                X = xpool.tile([P, CO, N], F32, tag="X")
                Xs[b] = X
                # first 128 columns land alone so transposes start early
                nc.sync.dma_start(X[:, :, 0:P], x_b[:, :, 0:P])
                nc.sync.dma_start(X[:, :, P:512], x_b[:, :, P:512])
                for nf in range(1, NF):
                    s = slice(nf * 512, (nf + 1) * 512)
                    nc.sync.dma_start(X[:, :, s], x_b[:, :, s])

            def emit_cast(b):
                """f32->bf16 natural-layout copy: matmul-2's moving
                operand and the bf16-transpose source. Emitted at the
                start of batch b's own cycle: the single X16 buffer is
                free right then (mm2(b-1) just finished reading it), so
                these never head-of-line block the DVE queue. Chunk
                order starts at bf_cut's chunk so the first bf16
                transposes of phase A unblock earliest."""
                X = Xs[b]
                X16 = x16pool.tile([P, CO, N], BF16, tag="X16")
                X16s[b] = X16
                first_chunk = o["bf_cut"] // 4
                order = [(first_chunk + i) % NF for i in range(NF)]
                # cast per (co, nf) piece: contiguous free dim -> DVE 2x mode
                for nf in order:
                    s = slice(nf * 512, (nf + 1) * 512)
                    for co in range(CO):
                        eng_copy(o["cast_engine"], X16[:, co, s], X[:, co, s])

            def emit_trans(b, kc, evac=None):
                """Transpose the kc-th 128-col slab of x_b into xt bf16.
                kc >= bf_cut reads the bf16 X16 (faster PE transpose);
                below that reads X f32 (no cast dependency -- used by the
                cross-batch filler and the first A-phase groups)."""
                src16 = kc >= o["bf_cut"]
                ks = slice(kc * P, (kc + 1) * P)
                src = X16s[b] if src16 else Xs[b]
                idt = ident16 if src16 else ident
                # bf16 tiles padded to a full 2KB PSUM bank (same slot
                # size as the f32 ones) so the two psx ring buffers never
                # share a bank (PE-W vs evac-R collide at bank granularity)
                ps_x = psum_xt.tile(
                    [P, C], BF16 if src16 else F32,
                    tag="psx", name="ps_x",
                    padded_shape=[P, 1024] if src16 else None,
                )
                for co in range(CO):
                    nc.tensor.transpose(
                        ps_x[:, co * P:(co + 1) * P], src[:, co, ks], idt
                    )
                xt_k = xtp.tile([P, C], BF16, tag="xt", name="xt_k")
                eng_copy(evac or o["evac_engine"], xt_k[:, :], ps_x[:, :])
                xts[(b, kc)] = xt_k

            def emit_mm1(b, kc, E):
                xt_k = xts.pop((b, kc))
                for ic in range(CO):
                    nc.tensor.matmul(
                        E[:, ic, ic * P:],
                        xt_k[:, ic * P:(ic + 1) * P],
                        xt_k[:, ic * P:],
                        start=(kc == 0),
                        stop=(kc == KC - 1),
                    )

            def emit_mirror(b, E):
                for jc in range(1, CO):
                    for ic in range(jc):
                        stg = xtp.tile([P, P], F32, tag="mirror_stage",
                                       bufs=1)
                        nc.scalar.copy(
                            stg[:, :], E[:, ic, jc * P:(jc + 1) * P]
                        )
                        nc.tensor.matmul(
                            E[:, jc, ic * P:(ic + 1) * P],
                            stg[:, :],
                            ident,
                            is_transpose=True,
                            skip_group_check=True,
                        )

            def emit_softmax(b, E):
                mn = stats.tile([P, CO], F32, tag="mn")
                zs = stats.tile([P, CO], F32, tag="zs")
                rg = stats.tile([P, CO], F32, tag="rg")
                tS = tpool.tile([P, CO, C], TSDT, tag="t")
                for ic in range(CO):
                    nc.vector.tensor_reduce(
                        mn[:, ic:ic + 1], E[:, ic, :], AX.X, ALU.min
                    )
                for ic in range(CO):
                    nc.scalar.activation(
                        tS[:, ic, :], E[:, ic, :], AF.Exp,
                        bias=mn[:, ic:ic + 1], scale=-1.0,
                        accum_out=zs[:, ic:ic + 1],
                    )
                nc.vector.reciprocal(rg[:, :], zs[:, :])
                nc.vector.tensor_scalar_mul(rg[:, :], rg[:, :], g_col[:, :1])
                return tS, rg

            def emit_tT(b, tS):
                tT = ttpool.tile([P, CO, C], BF16, tag="tT")
                idt = ident16 if o["ts_bf16"] else ident
                for jc in range(CO):
                    ps_t = psum_acc.tile(
                        [P, C], TSDT, tag="acc", name="ps_t",
                        padded_shape=[P, 1024] if o["ts_bf16"] else None,
                    )
                    for ic in range(CO):
                        nc.tensor.transpose(
                            ps_t[:, ic * P:(ic + 1) * P],
                            tS[:, ic, jc * P:(jc + 1) * P],
                            idt,
                        )
                    nc.scalar.copy(tT[:, jc, :], ps_t[:, :])
                return tT

            def emit_mm2(b, tT, rg):
                X, X16 = Xs[b], X16s[b]
                y_b = y_out[b].rearrange("(co p) n -> p co n", p=P)
                for ic in range(CO):
                    for nf in range(NF):
                        ns = slice(nf * 512, (nf + 1) * 512)
                        g = ic * NF + nf
                        if g % 3 == 2:
                            # E's PSUM region is dead during mm2 (softmax
                            # already read it); borrowing it as a third
                            # rotation slot hides the ScalarE evac latency
                            # behind two full matmul groups
                            ps2 = psum_e.tile([P, C], F32, tag="E",
                                              name="ps2e")
                        else:
                            ps2 = psum_acc.tile([P, C], F32, tag="acc")
                        for jc in range(CO):
                            nc.tensor.matmul(
                                ps2[:, :512],
                                tT[:, jc, ic * P:(ic + 1) * P],
                                X16[:, jc, ns],
                                start=(jc == 0),
                                stop=(jc == CO - 1),
                            )
                        ot = opool.tile([P, 512], F32, tag="o")
                        if o["evac_split"]:
                            nc.scalar.activation(
                                ot[:, :], ps2[:, :512], AF.Copy,
                                bias=0.0, scale=rg[:, ic:ic + 1],
                            )
                            nc.vector.tensor_add(
                                ot[:, :], ot[:, :], X[:, ic, ns]
                            )
                        else:
                            nc.vector.scalar_tensor_tensor(
                                ot[:, :], ps2[:, :512], rg[:, ic:ic + 1],
                                X[:, ic, ns],
                                op0=ALU.mult, op1=ALU.add,
                            )
                        nc.sync.dma_start(y_b[:, ic, ns], ot[:, :])
                del Xs[b], X16s[b]

            loop_ctx = (
                tc.For_i(0, reps, 1) if reps > 1 else contextlib.nullcontext()
            )
            with loop_ctx:
                emit_dma(0)
                for b in range(Bs):
                    first = (b == 0)
                    emit_cast(b)
                    E = psum_e.tile([P, CO, C], F32, tag="E")
                    if first:
                        # no prefetched transposes: run 2 ahead of mm1
                        emit_trans(b, 0)
                        emit_trans(b, 1)
                        for kc in range(KC):
                            if kc + 2 < KC:
                                emit_trans(b, kc + 2)
                            if kc == 16 and b + 1 < Bs:
                                emit_dma(b + 1)
                            emit_mm1(b, kc, E)
                    else:
                        # kc < PRE were transposed during softmax(b-1)
                        for kc in range(KC):
                            if kc == 0 and b + 1 < Bs:
                                emit_dma(b + 1)
                            if kc + PRE < KC:
                                emit_trans(b, kc + PRE)
                            emit_mm1(b, kc, E)
                    emit_mirror(b, E)
                    tS, rg = emit_softmax(b, E)
                    if b + 1 < Bs:
                        for kc in range(min(PRE, KC)):
                            emit_trans(b + 1, kc)
                    tT = emit_tT(b, tS)
                    emit_mm2(b, tT, rg)

    nc.compile()
    return nc


def get_nc(Bs=4, C=512, N=4096, use_f32r=False, reps=1, **opts):
    key = (Bs, C, N, use_f32r, reps, tuple(sorted(opts.items())))
    if key not in _CACHE:
        _CACHE[key] = _build(Bs, C, N, use_f32r, reps, **opts)
    return _CACHE[key]


def kernel(x, gamma):
    """Full inputs in, full output out. x [32, 512, 4096] f32, gamma [1] f32."""
    from concourse.bass_utils import run_bass_kernel_spmd

    x = np.ascontiguousarray(np.asarray(x, dtype=np.float32))
    gamma = np.ascontiguousarray(np.asarray(gamma, dtype=np.float32))
    B, C, N = x.shape
    n_cores = 8
    assert B % n_cores == 0
    Bs = B // n_cores

    nc = get_nc(Bs, C, N)
    in_maps = [
        {"x": x[i * Bs:(i + 1) * Bs], "gamma": gamma} for i in range(n_cores)
    ]
    res = run_bass_kernel_spmd(nc, in_maps, core_ids=list(range(n_cores)))
    return np.concatenate([r["y"] for r in res.results], axis=0)
